# revision 1
# baseline (speedup 1.0000x reference)
"""Plastic (Hebbian) FC layer — Trainium2 Bass kernel, 8 NeuronCores.

Problem: y_t = tanh(x_t @ (w + alpha*hebb_t)); hebb_{t+1} = (1-eta)*hebb_t
         + eta * outer(x_t, y_t), per example, T=128 steps, N=512.

Sharding: data-parallel over batch B=8 -> one example per core (the hebb
trace is per-example, so cores are fully independent; no collectives).

Per-core algorithm (time-blocked, all decay factors folded into
host-precomputed scale tables so on-chip work is pure matmul/FMA):
  q_s = eta*(1-eta)^(-1-s),  y_t = tanh(d^t * ACC_t)
  ACC_t = (x_t @ w) d^-t  +  x_t @ (alpha .* Htilde_b0)
          + sum_{s in block, s<t} ((x_t .* x_s q_s) @ alpha) .* y_s
  Htilde += sum_{s in block} (x_s q_s) y_s^T       (PSUM-accumulated)

All matmuls run in bf16 (fp32 matmul costs 4 cycles/row + 2x LDWEIGHTS);
the serial chain (tanh + FMA recurrence) stays fp32 with the block
accumulator living directly in the matmul's PSUM output.
"""

import sys

for _p in ("/opt/trn_rl_repo", "/opt/pypackages"):
    if _p not in sys.path:
        sys.path.insert(0, _p)

import numpy as np
import ml_dtypes

B, T, N = 8, 128, 512
TB = 16                 # time-block size
NB = T // TB            # number of blocks
NG = N // 128           # 4 column/row groups of 128
N_CORES = 8
BF16 = ml_dtypes.bfloat16


def _build(eta_f: float):
    import concourse.bass as bass
    import concourse.tile as tile
    from concourse import bacc, mybir

    f32 = mybir.dt.float32
    bf = mybir.dt.bfloat16
    d = 1.0 - eta_f
    g = [float(d**t) for t in range(T)]   # tanh scale immediates

    nc = bacc.Bacc(None, target_bir_lowering=False)

    xt_e = nc.declare_dram_parameter("xt", [128, NG, T], bf, isOutput=False)
    xtd_e = nc.declare_dram_parameter("xtd", [128, NG, T], bf, isOutput=False)
    xq_e = nc.declare_dram_parameter("xq", [128, NG, T], bf, isOutput=False)
    xh_e = nc.declare_dram_parameter("xh", [TB, NB, N], bf, isOutput=False)
    wm_e = nc.declare_dram_parameter("wm", [128, NG, N], bf, isOutput=False)
    ab_e = nc.declare_dram_parameter("ab", [128, NG, N], bf, isOutput=False)
    am_e = nc.declare_dram_parameter("am", [128, NG, N], f32, isOutput=False)
    id_e = nc.declare_dram_parameter("ident", [128, 128], f32, isOutput=False)
    yo_e = nc.declare_dram_parameter("yout", [128, NG, T], f32, isOutput=True)

    Tanh = None
    with tile.TileContext(nc) as tc:
        with (
            tc.tile_pool(name="persist", bufs=1) as pp,
            tc.tile_pool(name="blkbuf", bufs=2) as bp,
            tc.tile_pool(name="ps_ht", bufs=1, space=bass.MemorySpace.PSUM) as ps_ht,
            tc.tile_pool(name="ps_a", bufs=1, space=bass.MemorySpace.PSUM) as ps_a,
            tc.tile_pool(name="ps_bb", bufs=2, space=bass.MemorySpace.PSUM) as ps_bb,
        ):
            XT = pp.tile([128, NG, T], bf)
            XTD = pp.tile([128, NG, T], bf)
            XQ = pp.tile([128, NG, T], bf)
            XH = pp.tile([TB, NB, N], bf)
            WM = pp.tile([128, NG, N], bf)
            AB = pp.tile([128, NG, N], bf)       # alpha bf16 (A matmul lhsT)
            AM = pp.tile([128, NG, N], f32)      # alpha f32 (aeff multiply)
            IDT = pp.tile([128, 128], f32)
            AEFF = pp.tile([128, NG, N], bf)
            Y = pp.tile([128, NG, T], f32)
            TMP = pp.tile([128, NG, TB - 1], f32)
            HT = ps_ht.tile([128, NG, N], f32)   # 4 banks, lives all kernel

            nc.sync.dma_start(XT[:], xt_e[:])
            nc.sync.dma_start(XTD[:], xtd_e[:])
            nc.sync.dma_start(XQ[:], xq_e[:])
            nc.sync.dma_start(XH[:], xh_e[:])
            nc.sync.dma_start(WM[:], wm_e[:])
            nc.sync.dma_start(AB[:], ab_e[:])
            nc.sync.dma_start(AM[:], am_e[:])
            nc.sync.dma_start(IDT[:], id_e[:])

            Tanh = mybir.ActivationFunctionType.Tanh
            Copy = mybir.ActivationFunctionType.Copy
            mult = mybir.AluOpType.mult

            def make_pair(blk):
                # PAIR[ip, ig, tl, sl] = XT[:,ig,b0+tl] * XQ[:,ig,b0+sl]
                b0 = blk * TB
                P = bp.tile([128, NG, TB, TB], bf, tag="pair")
                op_t = XT[:, :, b0:b0 + TB].unsqueeze(3) \
                    .broadcast_to((128, NG, TB, TB))
                op_s = XQ[:, :, b0:b0 + TB].unsqueeze(2) \
                    .broadcast_to((128, NG, TB, TB))
                nc.vector.tensor_mul(P[:], op_t, op_s)
                return P

            PAIR = make_pair(0)
            for blk in range(NB):
                b0 = blk * TB
                if blk > 0:
                    # AEFF = alpha .* Htilde  (PSUM src, DVE, bf16 out)
                    nc.vector.tensor_mul(AEFF[:], AM[:], HT[:])

                # A[jp, jc, tl, sl] = sum_i alpha[i, jc*128+jp]*PAIR[i, tl, sl]
                APS = ps_a.tile([128, NG, TB, TB], f32, tag="aps")  # 2 banks
                for jc in range(NG):
                    for ig in range(NG):
                        nc.tensor.matmul(
                            APS[:, jc, :, :],
                            AB[:, ig, jc * 128:(jc + 1) * 128],
                            PAIR[:, ig, :, :],
                            start=(ig == 0), stop=(ig == NG - 1),
                        )
                ASB = bp.tile([128, NG, TB, TB], f32, tag="asb")
                nc.scalar.activation(ASB[:], APS[:], Copy)

                # BB = base/accumulator for the chain, lives in PSUM.
                # All matmuls of the bank's accumulation chain are emitted
                # consecutively (start once, stop once — a start=True marks
                # the whole 2KB PSUM bank pending-zero).
                BB = ps_bb.tile([128, NG, TB], f32, tag="bb")       # 1 bank
                n_mm = NG * NG * (2 if blk > 0 else 1)
                k = 0
                for jc in range(NG):
                    for ig in range(NG):
                        nc.tensor.matmul(
                            BB[:, jc, :],
                            WM[:, ig, jc * 128:(jc + 1) * 128],
                            XTD[:, ig, b0:b0 + TB],
                            start=(k == 0), stop=(k == n_mm - 1),
                        )
                        k += 1
                    if blk > 0:
                        for ig in range(NG):
                            nc.tensor.matmul(
                                BB[:, jc, :],
                                AEFF[:, ig, jc * 128:(jc + 1) * 128],
                                XT[:, ig, b0:b0 + TB],
                                start=(k == 0), stop=(k == n_mm - 1),
                            )
                            k += 1

                # serial chain: tanh reads BB (PSUM); FMA accumulates into BB
                for s in range(TB):
                    t = b0 + s
                    nc.scalar.activation(Y[:, :, t], BB[:, :, s], Tanh,
                                         scale=g[t])
                    if s < TB - 1:
                        r = TB - 1 - s
                        ybc = Y[:, :, t].unsqueeze(2) \
                            .broadcast_to((128, NG, r))
                        nc.vector.tensor_mul(TMP[:, :, :r],
                                             ASB[:, :, s + 1:, s], ybc)
                        nc.vector.tensor_add(BB[:, :, s + 1:],
                                             BB[:, :, s + 1:], TMP[:, :, :r])

                if blk < NB - 1:
                    PAIR = make_pair(blk + 1)
                    # Htilde += (x_s q_s) y_s^T over this block
                    YTP = ps_a.tile([TB, NG, 128], f32, tag="aps")
                    for jc in range(NG):
                        nc.tensor.transpose(
                            YTP[:, jc, :], Y[:, jc, b0:b0 + TB], IDT[:])
                    YTR = bp.tile([TB, NG, 128], bf, tag="ytr")
                    nc.scalar.activation(YTR[:], YTP[:], Copy)
                    for ic in range(NG):
                        for jc in range(NG):
                            nc.tensor.matmul(
                                HT[:, ic, jc * 128:(jc + 1) * 128],
                                XH[:, blk, ic * 128:(ic + 1) * 128],
                                YTR[:, jc, :],
                                start=(blk == 0 and jc == 0),
                                stop=(blk == NB - 2 and jc == NG - 1),
                                skip_group_check=True,
                            )

            nc.sync.dma_start(yo_e[:], Y[:])

    nc.compile()
    return nc


def kernel(x, w, alpha, eta, _trace=False, _trace_kwargs=None):
    from concourse.bass_utils import run_bass_kernel_spmd

    x = np.asarray(x, np.float32)
    w = np.asarray(w, np.float32)
    alpha = np.asarray(alpha, np.float32)
    eta_f = float(np.asarray(eta).reshape(-1)[0])

    d = 1.0 - eta_f
    t_idx = np.arange(T, dtype=np.float64)
    wscale = (d ** (-t_idx)).astype(np.float32)                # d^-t
    qscale = (eta_f * d ** (-1.0 - t_idx)).astype(np.float32)  # eta*d^(-1-s)

    def to_grp(m, dt=BF16):  # [T,N] (cols=i) -> [128, NG, T], i = ig*128+ip
        return np.ascontiguousarray(
            m.T.reshape(NG, 128, T).transpose(1, 0, 2)).astype(dt)

    def to_wgrp(m, dt=BF16):  # [N,N] -> [128, NG, N], i = ig*128+ip
        return np.ascontiguousarray(
            m.reshape(NG, 128, N).transpose(1, 0, 2)).astype(dt)

    wm = to_wgrp(w)
    ab = to_wgrp(alpha)
    am = to_wgrp(alpha, np.float32)
    ident = np.eye(128, dtype=np.float32)

    in_maps = []
    for b in range(B):
        xb = x[b]                                   # [T, N]
        in_maps.append({
            "xt": to_grp(xb),
            "xtd": to_grp(xb * wscale[:, None]),
            "xq": to_grp(xb * qscale[:, None]),
            "xh": np.ascontiguousarray(
                (xb * qscale[:, None]).reshape(NB, TB, N)
                .transpose(1, 0, 2)).astype(BF16),
            "wm": wm, "ab": ab, "am": am, "ident": ident,
        })

    nc = _build(eta_f)
    res = run_bass_kernel_spmd(
        nc, in_maps, list(range(N_CORES)),
        trace=_trace, **(_trace_kwargs or {}))

    out = np.empty((B, T, N), np.float32)
    for b in range(B):
        yo = res.results[b]["yout"]                 # [128, NG, T]
        out[b] = yo.transpose(2, 1, 0).reshape(T, N)
    if _trace:
        kernel.last_result = res
    return out



# revision 6
# speedup vs baseline: 1.0289x; 1.0289x over previous
"""Plastic (Hebbian) FC layer — Trainium2 Bass kernel, 8 NeuronCores.

Problem: y_t = tanh(x_t @ (w + alpha*hebb_t)); hebb_{t+1} = (1-eta)*hebb_t
         + eta * outer(x_t, y_t), per example, T=128 steps, N=512.

Sharding: data-parallel over batch B=8 -> one example per core (the hebb
trace is per-example, so cores are fully independent; no collectives).

Tanh-domain formulation (d = 1-eta, xg_t = d^t x_t, xq_s = eta d^(-1-s) x_s):
  y_t = tanh(BB_t)
  BB_t = x_t @ w  +  xg_t @ (alpha .* H_<t)  +  sum_{s<t} A[.,t,s] .* y_s
  A[j,t,s] = ((xg_t .* xq_s) @ alpha)[j],   H = sum_s xq_s y_s^T

Schedule (the serial tanh chain is the critical path; every block's prep
runs inside the previous block's chain window on off-path engine slots):
  - WBASE (x@w, all T) computed once into one persistent PSUM bank; the
    per-block alpha.*H matmuls accumulate into its column slices (open
    accumulation group, lagged one block: slice k+1 uses H through k-1).
  - block k -> k+1 coupling via Pool FMAs (CROSS buffer, "crossbulk"),
    within-block coupling via DVE eager FMAs; both use precomputed A.
  - ACT does only tanh + (chunked) PSUM->SBUF copies in tanh idle slots.
"""

import sys

for _p in ("/opt/trn_rl_repo", "/opt/pypackages"):
    if _p not in sys.path:
        sys.path.insert(0, _p)

import numpy as np
import ml_dtypes

B, T, N = 8, 128, 512
TB = 16                 # time-block size
NB = T // TB            # number of blocks
NG = N // 128           # 4 column/row groups of 128
N_CORES = 8
BF16 = ml_dtypes.bfloat16


def _build(eta_f: float):
    import concourse.bass as bass
    import concourse.tile as tile
    from concourse import bacc, mybir

    f32 = mybir.dt.float32
    bf = mybir.dt.bfloat16

    nc = bacc.Bacc(None, target_bir_lowering=False)

    xt_e = nc.declare_dram_parameter("xt", [128, NG, T], bf, isOutput=False)
    xg_e = nc.declare_dram_parameter("xg", [128, NG, T], bf, isOutput=False)
    xq_e = nc.declare_dram_parameter("xq", [128, NG, T], bf, isOutput=False)
    xh_e = nc.declare_dram_parameter("xh", [TB, NB, N], bf, isOutput=False)
    wm_e = nc.declare_dram_parameter("wm", [128, NG, N], bf, isOutput=False)
    ab_e = nc.declare_dram_parameter("ab", [128, NG, N], bf, isOutput=False)
    id_e = nc.declare_dram_parameter("ident", [128, 128], f32, isOutput=False)
    yo_e = nc.declare_dram_parameter("yout", [128, NG, T], f32, isOutput=True)

    with tile.TileContext(nc) as tc:
        with (
            tc.tile_pool(name="persist", bufs=1) as pp,
            tc.tile_pool(name="dbuf", bufs=2) as bp,
            tc.tile_pool(name="ps_wb", bufs=1, space=bass.MemorySpace.PSUM) as ps_wb,
            tc.tile_pool(name="ps_ht", bufs=1, space=bass.MemorySpace.PSUM) as ps_ht,
            tc.tile_pool(name="ps_a", bufs=1, space=bass.MemorySpace.PSUM) as ps_a,
            tc.tile_pool(name="ps_yt", bufs=1, space=bass.MemorySpace.PSUM) as ps_yt,
        ):
            XT = pp.tile([128, NG, T], bf)
            XG = pp.tile([128, NG, T], bf)
            XQ = pp.tile([128, NG, T], bf)
            XH = pp.tile([TB, NB, N], bf)
            WM = pp.tile([128, NG, N], bf)
            AB = pp.tile([128, NG, N], bf)
            IDT = pp.tile([128, 128], f32)
            Y = pp.tile([128, NG, T], f32)
            HTS = pp.tile([128, NG, N], f32)     # SBUF copy of H for Pool
            TMP = pp.tile([128, NG, TB - 1], f32)
            TMPX = pp.tile([128, NG, TB], f32)

            WBASE = ps_wb.tile([128, NG, T], f32)    # 1 bank, all kernel
            HT = ps_ht.tile([128, NG, N], f32)       # 4 banks, all kernel

            Tanh = mybir.ActivationFunctionType.Tanh
            Copy = mybir.ActivationFunctionType.Copy

            nc.sync.dma_start(XG[:], xg_e[:])
            nc.sync.dma_start(XQ[:], xq_e[:])
            nc.sync.dma_start(XT[:], xt_e[:])
            nc.sync.dma_start(WM[:], wm_e[:])
            nc.sync.dma_start(AB[:], ab_e[:])
            nc.sync.dma_start(XH[:], xh_e[:])
            nc.sync.dma_start(IDT[:], id_e[:])

            def make_pairw(blk, chunk=None):
                # PAIRW[ip, ig, tl, sl] = XG[:,ig,b0+tl] * XQ[:,ig,b0+sl]
                b0 = blk * TB
                if chunk is None:
                    P = bp.tile([128, NG, TB, TB], bf, tag="pw")
                    gs = slice(0, NG)
                else:
                    P = chunk[0]
                    gs = slice(chunk[1], chunk[1] + 1)
                op_t = XG[:, gs, b0:b0 + TB].unsqueeze(3) \
                    .broadcast_to((128, gs.stop - gs.start, TB, TB))
                op_s = XQ[:, gs, b0:b0 + TB].unsqueeze(2) \
                    .broadcast_to((128, gs.stop - gs.start, TB, TB))
                nc.vector.tensor_mul(P[:, gs] if chunk else P[:], op_t, op_s)
                return P

            def make_pairx(blk, chunk=None):
                # PAIRX[ip, ig, tl, sl] = XG[:,ig,(blk+1)*TB+tl]*XQ[:,ig,blk*TB+sl]
                b0 = blk * TB
                b1 = b0 + TB
                if chunk is None:
                    P = bp.tile([128, NG, TB, TB], bf, tag="px")
                    gs = slice(0, NG)
                else:
                    P = chunk[0]
                    gs = slice(chunk[1], chunk[1] + 1)
                op_t = XG[:, gs, b1:b1 + TB].unsqueeze(3) \
                    .broadcast_to((128, gs.stop - gs.start, TB, TB))
                op_s = XQ[:, gs, b0:b0 + TB].unsqueeze(2) \
                    .broadcast_to((128, gs.stop - gs.start, TB, TB))
                nc.vector.tensor_mul(P[:, gs] if chunk else P[:], op_t, op_s)
                return P

            def a_matmuls(PAIR):
                APS = ps_a.tile([128, NG, TB, TB], f32, tag="apsx")
                for jc in range(NG):
                    for ig in range(NG):
                        nc.tensor.matmul(
                            APS[:, jc, :, :],
                            AB[:, ig, jc * 128:(jc + 1) * 128],
                            PAIR[:, ig, :, :],
                            start=(ig == 0), stop=(ig == NG - 1),
                        )
                return APS

            def wbase_matmuls():
                first = True
                for jc in range(NG):
                    for ig in range(NG):
                        nc.tensor.matmul(
                            WBASE[:, jc, :],
                            WM[:, ig, jc * 128:(jc + 1) * 128],
                            XT[:, ig, :],
                            start=first, stop=False,
                            skip_group_check=True,
                        )
                        first = False

            def aeff_matmuls(AEFF, blk, last=False):
                # WBASE[:, :, blk cols] += XG_blk @ (alpha .* H)
                b0 = blk * TB
                for jc in range(NG):
                    for ig in range(NG):
                        nc.tensor.matmul(
                            WBASE[:, jc, b0:b0 + TB],
                            AEFF[:, ig, jc * 128:(jc + 1) * 128],
                            XG[:, ig, b0:b0 + TB],
                            start=False,
                            stop=(last and jc == NG - 1 and ig == NG - 1),
                            skip_group_check=True,
                        )

            # ---------- startup: prep block 0 (and cross 0->1) ----------
            PAIRW = make_pairw(0)
            APS = a_matmuls(PAIRW)
            ASB = bp.tile([128, NG, TB, TB], bf, tag="asb")
            nc.scalar.activation(ASB[:], APS[:], Copy)

            wbase_matmuls()
            BBS = bp.tile([128, NG, TB], f32, tag="bbs")
            nc.vector.tensor_copy(BBS[:], WBASE[:, :, 0:TB])

            PAIRX = make_pairx(0)
            APSX = a_matmuls(PAIRX)
            ASBX = bp.tile([128, NG, TB, TB], bf, tag="asbx")
            nc.scalar.activation(ASBX[:], APSX[:], Copy)

            # ---------- main loop ----------
            asbx_pending = None     # (ASBX_next, APSX_next) cols 8-15 copy
            for k in range(NB):
                b0 = k * TB
                prep = k < NB - 1        # prepare chain k+1
                hblk = 1 <= k <= NB - 2  # H/AEFF/aeff for slice k+1
                xblk = k < NB - 2        # prepare cross (k+1 -> k+2)

                if prep:
                    BBS_next = bp.tile([128, NG, TB], f32, tag="bbs")
                    CROSS = bp.tile([128, NG, TB], f32, tag="cross")
                    PAIRW_next = bp.tile([128, NG, TB, TB], bf, tag="pw")
                    ASB_next = bp.tile([128, NG, TB, TB], bf, tag="asb")
                if hblk:
                    YTP = ps_yt.tile([TB, NG, 128], f32, tag="ytp")
                    YTR = bp.tile([TB, NG, 128], bf, tag="ytr")
                    AEFF = bp.tile([128, NG, N], bf, tag="aeff")
                if xblk:
                    PAIRX_next = bp.tile([128, NG, TB, TB], bf, tag="px")
                    ASBX_next = bp.tile([128, NG, TB, TB], bf, tag="asbx")

                for s in range(TB):
                    t = b0 + s

                    # ---- serial chain ----
                    nc.scalar.activation(Y[:, :, t], BBS[:, :, s], Tanh)
                    if s < TB - 1:
                        r = TB - 1 - s
                        ybc = Y[:, :, t].unsqueeze(2).broadcast_to((128, NG, r))
                        nc.vector.tensor_mul(TMP[:, :, :r],
                                             ASB[:, :, s + 1:, s], ybc)
                        nc.vector.tensor_add(BBS[:, :, s + 1:],
                                             BBS[:, :, s + 1:], TMP[:, :, :r])

                    # ---- cross-block coupling k -> k+1 (Pool) ----
                    if prep:
                        ybc16 = Y[:, :, t].unsqueeze(2) \
                            .broadcast_to((128, NG, TB))
                        if s == 0:
                            nc.gpsimd.tensor_mul(CROSS[:], ASBX[:, :, :, 0],
                                                 ybc16)
                        elif s < TB - 1:
                            nc.gpsimd.tensor_mul(TMPX[:], ASBX[:, :, :, s],
                                                 ybc16)
                            nc.gpsimd.tensor_add(CROSS[:], CROSS[:], TMPX[:])
                        else:
                            nc.gpsimd.tensor_mul(TMPX[:], ASBX[:, :, :, s],
                                                 ybc16)
                            nc.gpsimd.tensor_add(BBS_next[:], BBS_next[:],
                                                 TMPX[:])

                    # ---- off-path prep in idle slots ----
                    if s in (0, 1) and asbx_pending is not None:
                        pA, pP = asbx_pending
                        c = 8 + (s % 2) * 4
                        nc.scalar.activation(pA[:, :, :, c:c + 4],
                                             pP[:, :, :, c:c + 4], Copy)
                        if s == 1:
                            asbx_pending = None
                    if s == 0 and hblk:
                        bprev = b0 - TB
                        for jc in range(NG):
                            nc.tensor.transpose(
                                YTP[:, jc, :], Y[:, jc, bprev:bprev + TB],
                                IDT[:])
                    if s in (0, 1) and hblk:
                        h = (s % 2) * 2
                        nc.vector.tensor_copy(YTR[:, h:h + 2, :],
                                              YTP[:, h:h + 2, :])
                    if 2 <= s <= 5 and prep:
                        make_pairw(k + 1, chunk=(PAIRW_next, s - 2))
                    if s == 4 and hblk:
                        for ic in range(NG):
                            nc.tensor.matmul(
                                HT[:, ic, :],
                                XH[:, k - 1, ic * 128:(ic + 1) * 128],
                                YTR[:, :, :],
                                start=(k == 1),
                                stop=(k == NB - 2),
                                skip_group_check=True,
                            )
                    if s in (5, 6, 7, 12) and hblk:
                        ic = {5: 0, 6: 1, 7: 2, 12: 3}[s]
                        nc.scalar.activation(HTS[:, ic, :], HT[:, ic, :],
                                             Copy)
                    if s in (7, 9, 11, 13) and hblk:
                        ic = {7: 0, 9: 1, 11: 2, 13: 3}[s]
                        nc.gpsimd.tensor_mul(AEFF[:, ic, :], AB[:, ic, :],
                                             HTS[:, ic, :])
                    if 6 <= s <= 9 and xblk:
                        make_pairx(k + 1, chunk=(PAIRX_next, s - 6))
                    if s == 6 and prep:
                        APS_next = a_matmuls(PAIRW_next)
                    if 8 <= s <= 11 and prep:
                        c = (s - 8) * 4
                        nc.scalar.activation(
                            ASB_next[:, :, c:c + 4, :],
                            APS_next[:, :, c:c + 4, :], Copy)
                    if s == 12 and xblk:
                        APSX_next = a_matmuls(PAIRX_next)
                    if s == 14 and hblk:
                        aeff_matmuls(AEFF, k + 1, last=(k == NB - 2))
                    if s == 14 and prep:
                        # boundary: BBS(k+1) = WBASE slice + CROSS
                        nc.vector.tensor_add(
                            BBS_next[:], WBASE[:, :, b0 + TB:b0 + 2 * TB],
                            CROSS[:])
                    if s in (14, 15) and xblk:
                        c = (s - 14) * 4
                        nc.scalar.activation(
                            ASBX_next[:, :, :, c:c + 4],
                            APSX_next[:, :, :, c:c + 4], Copy)

                # ---- end of window: output DMA ----
                nc.sync.dma_start(yo_e[:, :, b0:b0 + TB], Y[:, :, b0:b0 + TB])
                if xblk:
                    asbx_pending = (ASBX_next, APSX_next)

                if prep:
                    BBS = BBS_next
                    ASB = ASB_next
                if xblk:
                    ASBX = ASBX_next

    nc.compile()
    return nc


def kernel(x, w, alpha, eta, _trace=False, _trace_kwargs=None):
    from concourse.bass_utils import run_bass_kernel_spmd

    x = np.asarray(x, np.float32)
    w = np.asarray(w, np.float32)
    alpha = np.asarray(alpha, np.float32)
    eta_f = float(np.asarray(eta).reshape(-1)[0])

    d = 1.0 - eta_f
    t_idx = np.arange(T, dtype=np.float64)
    gscale = (d ** t_idx).astype(np.float32)                   # d^t
    qscale = (eta_f * d ** (-1.0 - t_idx)).astype(np.float32)  # eta*d^(-1-s)

    def to_grp(m, dt=BF16):  # [T,N] (cols=i) -> [128, NG, T], i = ig*128+ip
        return np.ascontiguousarray(
            m.T.reshape(NG, 128, T).transpose(1, 0, 2)).astype(dt)

    def to_wgrp(m, dt=BF16):  # [N,N] -> [128, NG, N], i = ig*128+ip
        return np.ascontiguousarray(
            m.reshape(NG, 128, N).transpose(1, 0, 2)).astype(dt)

    wm = to_wgrp(w)
    ab = to_wgrp(alpha)
    ident = np.eye(128, dtype=np.float32)

    in_maps = []
    for b in range(B):
        xb = x[b]                                   # [T, N]
        in_maps.append({
            "xt": to_grp(xb),
            "xg": to_grp(xb * gscale[:, None]),
            "xq": to_grp(xb * qscale[:, None]),
            "xh": np.ascontiguousarray(
                (xb * qscale[:, None]).reshape(NB, TB, N)
                .transpose(1, 0, 2)).astype(BF16),
            "wm": wm, "ab": ab, "ident": ident,
        })

    nc = _build(eta_f)
    res = run_bass_kernel_spmd(
        nc, in_maps, list(range(N_CORES)),
        trace=_trace, **(_trace_kwargs or {}))

    out = np.empty((B, T, N), np.float32)
    for b in range(B):
        yo = res.results[b]["yout"]                 # [128, NG, T]
        out[b] = yo.transpose(2, 1, 0).reshape(T, N)
    if _trace:
        kernel.last_result = res
    return out


# revision 14
# speedup vs baseline: 1.1476x; 1.1154x over previous
"""Plastic (Hebbian) FC layer — Trainium2 Bass kernel, 8 NeuronCores.

Problem: y_t = tanh(x_t @ (w + alpha*hebb_t)); hebb_{t+1} = (1-eta)*hebb_t
         + eta * outer(x_t, y_t), per example, T=128 steps, N=512.

Sharding: data-parallel over batch B=8 -> one example per core (the hebb
trace is per-example, so cores are fully independent; no collectives).

Tanh-domain formulation (d = 1-eta, xg_t = d^t x_t, xq_s = eta d^(-1-s) x_s):
  y_t = tanh(BB_t)
  BB_t = x_t @ w  +  xg_t @ (alpha .* H_<t)  +  sum_{s<t} A[.,t,s] .* y_s
  A[j,t,s] = ((xg_t .* xq_s) @ alpha)[j],   H = sum_s xq_s y_s^T

Schedule (the serial tanh chain is the critical path; every block's prep
runs inside the previous block's chain window on off-path engine slots):
  - WBASE (x@w, all T) computed once into one persistent PSUM bank; the
    per-block alpha.*H matmuls accumulate into its column slices (open
    accumulation group, lagged one block: slice k+1 uses H through k-1).
  - block k -> k+1 coupling via Pool FMAs (CROSS buffer, "crossbulk"),
    within-block coupling via DVE eager FMAs; both use precomputed A.
  - ACT does only tanh + (chunked) PSUM->SBUF copies in tanh idle slots.
"""

import sys

for _p in ("/opt/trn_rl_repo", "/opt/pypackages"):
    if _p not in sys.path:
        sys.path.insert(0, _p)

import numpy as np
import ml_dtypes

B, T, N = 8, 128, 512
TB = 16                 # time-block size
NB = T // TB            # number of blocks
NG = N // 128           # 4 column/row groups of 128
N_CORES = 8
BF16 = ml_dtypes.bfloat16


def _build(eta_f: float):
    import concourse.bass as bass
    import concourse.tile as tile
    from concourse import bacc, mybir

    f32 = mybir.dt.float32
    bf = mybir.dt.bfloat16

    nc = bacc.Bacc(None, target_bir_lowering=False)

    # packed inputs: 3 staged DMAs so early consumers start ASAP
    xgq_e = nc.declare_dram_parameter("xgq", [128, NG, 2 * T], bf,
                                      isOutput=False)   # xg | xq
    ab_e = nc.declare_dram_parameter("ab", [128, NG, N], bf, isOutput=False)
    twx_e = nc.declare_dram_parameter("twx", [128, NG, T + N], bf,
                                      isOutput=False)   # xt | wm
    xh_e = nc.declare_dram_parameter("xh", [TB, NB, N], bf, isOutput=False)
    id_e = nc.declare_dram_parameter("ident", [128, 128], f32, isOutput=False)
    yo_e = nc.declare_dram_parameter("yout", [128, NG, T], f32, isOutput=True)

    with tile.TileContext(nc) as tc:
        with (
            tc.tile_pool(name="persist", bufs=1) as pp,
            tc.tile_pool(name="dbuf", bufs=2) as bp,
            tc.tile_pool(name="ps_wb", bufs=1, space=bass.MemorySpace.PSUM) as ps_wb,
            tc.tile_pool(name="ps_ht", bufs=1, space=bass.MemorySpace.PSUM) as ps_ht,
            tc.tile_pool(name="ps_a", bufs=1, space=bass.MemorySpace.PSUM) as ps_a,
            tc.tile_pool(name="ps_yt", bufs=1, space=bass.MemorySpace.PSUM) as ps_yt,
        ):
            XGQ = pp.tile([128, NG, 2 * T], bf)      # xg | xq packed
            TWX = pp.tile([128, NG, T + N], bf)      # xt | wm packed
            XH = pp.tile([TB, NB, N], bf)
            AB = pp.tile([128, NG, N], bf)
            IDT = pp.tile([128, 128], f32)
            Y = pp.tile([128, NG, T], f32)
            HTS = pp.tile([128, NG, N], bf)      # SBUF copy of H (bf16)
            TMP = pp.tile([128, NG, TB - 1], f32)
            TMPX = pp.tile([128, NG, TB], bf)
            XG = XGQ[:, :, :T]
            XQ = XGQ[:, :, T:]
            XT = TWX[:, :, :T]
            WM = TWX[:, :, T:]

            WBASE = ps_wb.tile([128, NG, T], f32)    # 1 bank, all kernel
            HT = ps_ht.tile([128, NG, N], f32)       # 4 banks, all kernel

            Tanh = mybir.ActivationFunctionType.Tanh
            Copy = mybir.ActivationFunctionType.Copy

            nc.sync.dma_start(XGQ[:], xgq_e[:])
            nc.sync.dma_start(AB[:], ab_e[:])
            nc.sync.dma_start(TWX[:], twx_e[:])
            nc.sync.dma_start(XH[:], xh_e[:])
            nc.sync.dma_start(IDT[:], id_e[:])

            def make_pairw(blk, chunk=None):
                # PAIRW[ip, ig, tl, sl] = XG[:,ig,b0+tl] * XQ[:,ig,b0+sl]
                b0 = blk * TB
                if chunk is None:
                    P = bp.tile([128, NG, TB, TB], bf, tag="pw")
                    gs = slice(0, NG)
                else:
                    P = chunk[0]
                    gs = slice(chunk[1], chunk[1] + 1)
                op_t = XG[:, gs, b0:b0 + TB].unsqueeze(3) \
                    .broadcast_to((128, gs.stop - gs.start, TB, TB))
                op_s = XQ[:, gs, b0:b0 + TB].unsqueeze(2) \
                    .broadcast_to((128, gs.stop - gs.start, TB, TB))
                nc.vector.tensor_mul(P[:, gs] if chunk else P[:], op_t, op_s)
                return P

            def make_pairx(blk, chunk=None):
                # PAIRX[ip, ig, tl, sl] = XG[:,ig,(blk+1)*TB+tl]*XQ[:,ig,blk*TB+sl]
                b0 = blk * TB
                b1 = b0 + TB
                if chunk is None:
                    P = bp.tile([128, NG, TB, TB], bf, tag="px")
                    gs = slice(0, NG)
                else:
                    P = chunk[0]
                    gs = slice(chunk[1], chunk[1] + 1)
                op_t = XG[:, gs, b1:b1 + TB].unsqueeze(3) \
                    .broadcast_to((128, gs.stop - gs.start, TB, TB))
                op_s = XQ[:, gs, b0:b0 + TB].unsqueeze(2) \
                    .broadcast_to((128, gs.stop - gs.start, TB, TB))
                nc.vector.tensor_mul(P[:, gs] if chunk else P[:], op_t, op_s)
                return P

            def a_matmuls(PAIR):
                APS = ps_a.tile([128, NG, TB, TB], f32, tag="apsx")
                for jc in range(NG):
                    for ig in range(NG):
                        nc.tensor.matmul(
                            APS[:, jc, :, :],
                            AB[:, ig, jc * 128:(jc + 1) * 128],
                            PAIR[:, ig, :, :],
                            start=(ig == 0), stop=(ig == NG - 1),
                        )
                return APS

            def wbase_matmuls():
                first = True
                for jc in range(NG):
                    for ig in range(NG):
                        nc.tensor.matmul(
                            WBASE[:, jc, :],
                            WM[:, ig, jc * 128:(jc + 1) * 128],
                            XT[:, ig, :],
                            start=first, stop=False,
                            skip_group_check=True,
                        )
                        first = False

            def aeff_matmuls(AEFF, blk, last=False):
                # WBASE[:, :, blk cols] += XG_blk @ (alpha .* H)
                b0 = blk * TB
                for jc in range(NG):
                    for ig in range(NG):
                        nc.tensor.matmul(
                            WBASE[:, jc, b0:b0 + TB],
                            AEFF[:, ig, jc * 128:(jc + 1) * 128],
                            XG[:, ig, b0:b0 + TB],
                            start=False,
                            stop=(last and jc == NG - 1 and ig == NG - 1),
                            skip_group_check=True,
                        )

            # ---------- startup: prep block 0 (and cross 0->1) ----------
            PAIRW = make_pairw(0)
            PAIRX = make_pairx(0)
            APS = a_matmuls(PAIRW)
            ASB = bp.tile([128, NG, TB, TB], bf, tag="asb")
            nc.scalar.activation(ASB[:], APS[:], Copy)

            wbase_matmuls()
            BBS = bp.tile([128, NG, TB], f32, tag="bbs")
            nc.vector.tensor_copy(BBS[:], WBASE[:, :, 0:TB])

            APSX = a_matmuls(PAIRX)
            ASBX = bp.tile([128, NG, TB, TB], bf, tag="asbx")
            nc.scalar.activation(ASBX[:], APSX[:], Copy)

            # ---------- main loop ----------
            # ASBX source-col chunks still to copy, fired in s0.. slots
            asbx_pending = None
            for k in range(NB):
                b0 = k * TB
                prep = k < NB - 1        # prepare chain k+1
                hblk = 1 <= k <= NB - 2  # H/AEFF/aeff for slice k+1
                xblk = k < NB - 2        # prepare cross (k+1 -> k+2)

                if prep:
                    BBS_next = bp.tile([128, NG, TB], f32, tag="bbs")
                    CROSS = bp.tile([128, NG, TB], f32, tag="cross")
                    PAIRW_next = bp.tile([128, NG, TB, TB], bf, tag="pw")
                    ASB_next = bp.tile([128, NG, TB, TB], bf, tag="asb")
                if hblk:
                    YTP = ps_yt.tile([TB, NG, 128], f32, tag="ytp")
                    YTR = bp.tile([TB, NG, 128], bf, tag="ytr")
                    AEFF = bp.tile([128, NG, N], bf, tag="aeff")
                if xblk:
                    PAIRX_next = bp.tile([128, NG, TB, TB], bf, tag="px")
                    ASBX_next = bp.tile([128, NG, TB, TB], bf, tag="asbx")

                for s in range(TB):
                    t = b0 + s

                    # ---- serial chain ----
                    nc.scalar.activation(Y[:, :, t], BBS[:, :, s], Tanh)
                    if s < TB - 1:
                        r = TB - 1 - s
                        ybc = Y[:, :, t].unsqueeze(2).broadcast_to((128, NG, r))
                        nc.vector.tensor_mul(TMP[:, :, :r],
                                             ASB[:, :, s + 1:, s], ybc)
                        nc.vector.tensor_add(BBS[:, :, s + 1:],
                                             BBS[:, :, s + 1:], TMP[:, :, :r])

                    # ---- cross-block coupling k -> k+1 (Pool) ----
                    if prep:
                        ybc16 = Y[:, :, t].unsqueeze(2) \
                            .broadcast_to((128, NG, TB))
                        if s == 0:
                            nc.gpsimd.tensor_mul(CROSS[:], ASBX[:, :, :, 0],
                                                 ybc16)
                        elif s < TB - 1:
                            nc.gpsimd.tensor_mul(TMPX[:], ASBX[:, :, :, s],
                                                 ybc16)
                            nc.gpsimd.tensor_add(CROSS[:], CROSS[:], TMPX[:])
                        else:
                            nc.gpsimd.tensor_mul(TMPX[:], ASBX[:, :, :, s],
                                                 ybc16)
                            nc.gpsimd.tensor_add(BBS_next[:], BBS_next[:],
                                                 TMPX[:])

                    # ---- off-path prep in idle slots ----
                    if asbx_pending is not None and s < len(asbx_pending[2]):
                        pA, pP, cols = asbx_pending
                        c = cols[s]
                        nc.scalar.activation(pA[:, :, :, c:c + 4],
                                             pP[:, :, :, c:c + 4], Copy)
                        if s == len(cols) - 1:
                            asbx_pending = None
                    if s == 0 and hblk:
                        bprev = b0 - TB
                        for jc in range(NG):
                            nc.tensor.transpose(
                                YTP[:, jc, :], Y[:, jc, bprev:bprev + TB],
                                IDT[:])
                    if s in (0, 1) and hblk:
                        h = (s % 2) * 2
                        nc.vector.tensor_copy(YTR[:, h:h + 2, :],
                                              YTP[:, h:h + 2, :])
                    if 2 <= s <= 5 and prep:
                        make_pairw(k + 1, chunk=(PAIRW_next, s - 2))
                    if s == 2 and hblk:
                        for ic in range(NG):
                            nc.tensor.matmul(
                                HT[:, ic, :],
                                XH[:, k - 1, ic * 128:(ic + 1) * 128],
                                YTR[:, :, :],
                                start=(k == 1),
                                stop=(k == NB - 2),
                                skip_group_check=True,
                            )
                    if 3 <= s <= 10 and hblk:
                        ic, h = divmod(s - 3, 2)
                        nc.scalar.activation(
                            HTS[:, ic, h * 256:(h + 1) * 256],
                            HT[:, ic, h * 256:(h + 1) * 256], Copy)
                    if 6 <= s <= 8 and xblk:
                        c = (s - 6)
                        gs = [(0, 1), (1, 3), (3, 4)][c]
                        for g in range(gs[0], gs[1]):
                            make_pairx(k + 1, chunk=(PAIRX_next, g))
                    if s == 6 and prep:
                        APS_next = a_matmuls(PAIRW_next)
                    if 9 <= s <= 12 and hblk:
                        ic = s - 9
                        nc.vector.tensor_mul(AEFF[:, ic, :], AB[:, ic, :],
                                             HTS[:, ic, :])
                    if s in (11, 12) and prep:
                        c = (s - 11) * 8
                        nc.scalar.activation(
                            ASB_next[:, :, c:c + 8, :],
                            APS_next[:, :, c:c + 8, :], Copy)
                    if s == 12 and xblk:
                        APSX_next = a_matmuls(PAIRX_next)
                    if s == 13 and hblk:
                        aeff_matmuls(AEFF, k + 1, last=(k == NB - 2))
                    if s == 14 and prep:
                        # boundary: BBS(k+1) = WBASE slice + CROSS
                        nc.vector.tensor_add(
                            BBS_next[:], WBASE[:, :, b0 + TB:b0 + 2 * TB],
                            CROSS[:])
                    if s in (14, 15) and xblk:
                        c = (s - 14) * 4
                        nc.scalar.activation(
                            ASBX_next[:, :, :, c:c + 4],
                            APSX_next[:, :, :, c:c + 4], Copy)

                if xblk:
                    asbx_pending = (ASBX_next, APSX_next, [8, 12])

                if prep:
                    BBS = BBS_next
                    ASB = ASB_next
                if xblk:
                    ASBX = ASBX_next

            nc.sync.dma_start(yo_e[:], Y[:])

    nc.compile()
    return nc


def kernel(x, w, alpha, eta, _trace=False, _trace_kwargs=None):
    from concourse.bass_utils import run_bass_kernel_spmd

    x = np.asarray(x, np.float32)
    w = np.asarray(w, np.float32)
    alpha = np.asarray(alpha, np.float32)
    eta_f = float(np.asarray(eta).reshape(-1)[0])

    d = 1.0 - eta_f
    t_idx = np.arange(T, dtype=np.float64)
    gscale = (d ** t_idx).astype(np.float32)                   # d^t
    qscale = (eta_f * d ** (-1.0 - t_idx)).astype(np.float32)  # eta*d^(-1-s)

    def to_grp(m, dt=BF16):  # [T,N] (cols=i) -> [128, NG, T], i = ig*128+ip
        return np.ascontiguousarray(
            m.T.reshape(NG, 128, T).transpose(1, 0, 2)).astype(dt)

    def to_wgrp(m, dt=BF16):  # [N,N] -> [128, NG, N], i = ig*128+ip
        return np.ascontiguousarray(
            m.reshape(NG, 128, N).transpose(1, 0, 2)).astype(dt)

    wm = to_wgrp(w)
    ab = to_wgrp(alpha)
    ident = np.eye(128, dtype=np.float32)

    in_maps = []
    for b in range(B):
        xb = x[b]                                   # [T, N]
        xg = to_grp(xb * gscale[:, None])
        xq = to_grp(xb * qscale[:, None])
        xt = to_grp(xb)
        in_maps.append({
            "xgq": np.ascontiguousarray(
                np.concatenate([xg, xq], axis=2)),
            "twx": np.ascontiguousarray(
                np.concatenate([xt, wm], axis=2)),
            "xh": np.ascontiguousarray(
                (xb * qscale[:, None]).reshape(NB, TB, N)
                .transpose(1, 0, 2)).astype(BF16),
            "ab": ab, "ident": ident,
        })

    nc = _build(eta_f)
    res = run_bass_kernel_spmd(
        nc, in_maps, list(range(N_CORES)),
        trace=_trace, **(_trace_kwargs or {}))

    out = np.empty((B, T, N), np.float32)
    for b in range(B):
        yo = res.results[b]["yout"]                 # [128, NG, T]
        out[b] = yo.transpose(2, 1, 0).reshape(T, N)
    if _trace:
        kernel.last_result = res
    return out
